# revision 35
# baseline (speedup 1.0000x reference)
"""Causal multi-head attention on 8 trn2 NeuronCores (Megatron-style head parallelism).

Problem: B=2, L=2048, D=1024, H=16 heads (HD=64), fp32 in/out.

Sharding: each of the 8 cores owns 2 heads (a 128-wide slice of the QKV
projection output / Wo rows). Every core reads the full x; QKV projections are
column-sharded, attention runs per-head, the output projection is row-sharded
producing a partial f16 sum per core which the host reduces in f32 (+ bias).

On-chip layout: activations are kept feature-major ("transposed"):
  x^T [D, B*L] (host pre-transposes), Q^T/K^T/V^T [128(d), L] per batch.
Scores are computed transposed: S^T[k, q] = K^T_blk.T @ Q^T (contraction over
the head dim on partitions), both heads side by side in one [128,2,512] psum
tile so a single exp (and, on diagonal blocks, a single mask multiply) covers
them. Columns of diagonal blocks that are fully below the causal mask are
skipped in the scores matmul and the exp (strided AP over both heads); the
full-width mask multiply zeroes whatever stale data those columns hold (the
probs ring slots are memset once at startup so the stale data is never NaN).

Softmax runs along partitions via a ones-column appended to the V stationary
operands: each head's ctx accumulates into psum rows 0:65 of its own tile,
denominator in row 64. Both denominator rows are copied into partitions 0/64
of one staging tile so a single reciprocal_approx_fast serves both heads (DVE
time scales with free size only; a full InstReciprocal on a [1,512] row is
~3.3us, this path is ~0.7us). The reciprocal row is rounded to f32r and
broadcast across 64 partitions with a rank-1 f32r matmul; the raw-ctx
psum->sbuf copies run concurrently on DVE so the final normalize multiply
waits on neither. Head 0's normalized ctx lands on partitions 0:64 of a
packed [128(d), L] f16 tile directly; head 1 (born on partitions 0:64 -
matmul outputs can only start at partition 0/32/64) is shifted to partitions
64:128 by a small SBUF->SBUF DMA. The output projection then contracts over
all 128 dims in one matmul per 512 output columns, staged to f16 and DMA'd
out per 128-token block. The V bias never exists on device: softmax rows sum
to 1, so its contribution is the constant row bv @ Wo, folded into the host
bias add.

Matmul operands are float16 (10-bit mantissa ~ fp32r accuracy, bf16-class
speed); PSUM accumulation is fp32. exp needs no max-subtraction: |scores/8| <
~6 in fp32.

Scheduling notes (the PE only reaches its 2.4 GHz p-state after ~3us without
stalls; gaps drop it to 1.2 GHz):
- ~8us of framework preamble passes before any DMA moves; constants are
  issued on the Activation engine's DGE queue so the x stream starts on the
  Sync queue at t~=0 and the first projection matmul runs as early as
  possible. Weights are host-pre-arranged into their SBUF layouts so their
  DMAs are contiguous 2KB lines instead of 256B gather packets.
- attention on a query tile is emitted as soon as its key blocks exist
  (after half the projection strips), and batch-1 projections are emitted
  inside batch-0's attention so the shared PSUM rings pipeline across phases.
- PSUM budget (8 banks): scores 2x[128,1024] (4) + ctx 2x[128,512] (2) +
  transpose/broadcast/outproj ring 2x[128,512] (2).
- GPSIMD cannot touch PSUM and has ~1.2-1.9us per-op overhead, so it only
  does the startup memsets. Output staging copies run on DVE, except in the
  drain (batch 1, last two query tiles) where half go to the then-idle ACT.
"""

import numpy as np

_B, _L, _D, _H, _HD = 2, 2048, 1024, 16, 64
_NC = 8
_DC = _D // _NC          # 128 feature dims (2 heads) per core
_T = _B * _L             # 4096 tokens
_NKB = _L // 128         # 16 key blocks per batch
_NQT = _L // 512         # 4 query tiles per batch

_cache = {}


def _build_bass():
    from concourse import bacc
    import concourse.mybir as mybir
    import concourse.tile as tile

    f32 = mybir.dt.float32
    f32r = mybir.dt.float32r
    f16 = mybir.dt.float16
    AFT = mybir.ActivationFunctionType

    nc = bacc.Bacc("TRN2", target_bir_lowering=False, debug=False, num_devices=_NC)

    xT = nc.dram_tensor("xT", [_D, _T], f16, kind="ExternalInput")
    wq = nc.dram_tensor("wq", [128, 8, 128], f16, kind="ExternalInput")
    wk = nc.dram_tensor("wk", [128, 8, 128], f16, kind="ExternalInput")
    wv = nc.dram_tensor("wv", [128, 8, 128], f16, kind="ExternalInput")
    wo = nc.dram_tensor("wo", [_DC, _D], f16, kind="ExternalInput")
    bqd = nc.dram_tensor("bq", [_DC, 1], f32, kind="ExternalInput")
    bkd = nc.dram_tensor("bk", [_DC, 1], f32, kind="ExternalInput")
    msk = nc.dram_tensor("msk", [128, 4, 2, 512], f16, kind="ExternalInput")
    idn = nc.dram_tensor("idn", [128, 128], f16, kind="ExternalInput")
    ons = nc.dram_tensor("ons", [128, 64], f32r, kind="ExternalInput")
    onsb = nc.dram_tensor("onsb", [128, _NKB], f16, kind="ExternalInput")
    out = nc.dram_tensor("out", [_T, _D], f16, kind="ExternalOutput")

    with tile.TileContext(nc) as tc:
        with (
            tc.tile_pool(name="const", bufs=1) as constp,
            tc.tile_pool(name="xt", bufs=12) as xtp,
            tc.tile_pool(name="qkv", bufs=2) as qkvp,
            tc.tile_pool(name="probs", bufs=8) as probsp,
            tc.tile_pool(name="stage", bufs=2) as stagep,
            tc.tile_pool(name="ostg", bufs=6) as ostgp,
            tc.tile_pool(name="sc", bufs=2, space="PSUM") as scp,   # [128,1024] f32 = 2 banks
            tc.tile_pool(name="cx", bufs=2, space="PSUM") as cxp,   # [128,512] f32 = 1 bank
            tc.tile_pool(name="op", bufs=2, space="PSUM") as opp,   # [128,512] slot = 1 bank
        ):
            # ---- constants, group 1: needed by the first projection strip ----
            wq_sb = constp.tile([128, 8, 128], f16, tag="wq")
            bq_sb = constp.tile([128, 1], f32, tag="bq")
            wk_sb = constp.tile([128, 8, 128], f16, tag="wk")
            bk_sb = constp.tile([128, 1], f32, tag="bk")
            wv_sb = constp.tile([128, 8, 128], f16, tag="wv")
            nc.scalar.dma_start(wk_sb[:], wk[:])
            nc.scalar.dma_start(bk_sb[:], bkd[:])
            nc.scalar.dma_start(wq_sb[:], wq[:])
            nc.scalar.dma_start(bq_sb[:], bqd[:])
            nc.scalar.dma_start(wv_sb[:], wv[:])
            # group 2 (emitted after batch 0's x DMAs so x streams in early)
            idn_sb = constp.tile([128, 128], f16, tag="idn")
            onsb_sb = constp.tile([128, _NKB], f16, tag="onsb")
            msk_sb = constp.tile([128, 4, 2, 512], f16, tag="msk")
            ons_sb = constp.tile([128, 64], f32r, tag="ons")
            wo_sb = constp.tile([128, 1024], f16, tag="wo")

            # zero the probs ring slots once: narrowed diagonal exps leave
            # stale bytes in the skipped columns, and uninitialized SBUF can
            # hold NaN patterns that survive the mask multiply (NaN*0=NaN).
            for _i in range(8):
                przt = probsp.tile([128, 2, 512], f16, tag="pr",
                                   name=f"prz{_i}")
                nc.gpsimd.memset(przt[:], 0.0)

            st = {}  # per-batch SBUF tiles

            def emit_group2_consts():
                nc.scalar.dma_start(idn_sb[:], idn[:])
                nc.scalar.dma_start(onsb_sb[:], onsb[:])
                nc.scalar.dma_start(msk_sb[:], msk[:])
                nc.scalar.dma_start(ons_sb[:], ons[:])
                nc.scalar.dma_start(wo_sb[:], wo[:])

            def emit_x_load(b, tb2):
                t0 = b * _L + tb2 * 1024
                if tb2 == 0:
                    st[b] = {
                        "qT": qkvp.tile([128, _L], f16, tag="qT", name="qT"),
                        "kT": qkvp.tile([128, _L], f16, tag="kT", name="kT"),
                        "vT": qkvp.tile([128, _L], f16, tag="vT", name="vT"),
                        "ctx": qkvp.tile([128, _L], f16, tag="ctx", name="ctx"),
                        "xts": {},
                    }
                tiles = []
                for ec in range(8):
                    xt_t = xtp.tile([128, 1024], f16, tag="xt", name=f"xt{ec}")
                    nc.sync.dma_start(
                        xt_t[:], xT[ec * 128:(ec + 1) * 128, t0:t0 + 1024]
                    )
                    tiles.append(xt_t)
                st[b]["xts"][tb2] = tiles

            def emit_proj_strip(b, tb2):
                xts = st[b]["xts"][tb2]
                for w_sb, b_sb, dkey in (
                    (wk_sb, bk_sb, "kT"),
                    (wq_sb, bq_sb, "qT"),
                    (wv_sb, None, "vT"),
                ):
                    dst = st[b][dkey]
                    ps = scp.tile([128, 1024], f32, tag="sc")
                    for half in range(2):
                        col = half * 512
                        for ec in range(8):
                            nc.tensor.matmul(
                                ps[:, half * 512:(half + 1) * 512],
                                w_sb[:, ec, :],
                                xts[ec][:, col:col + 512],
                                start=(ec == 0),
                                stop=(ec == 7),
                            )
                    dcol = tb2 * 1024
                    if b_sb is not None:
                        nc.vector.tensor_scalar_add(
                            dst[:, dcol:dcol + 1024], ps[:], b_sb[:]
                        )
                    else:
                        # V needs no bias on device (folded into host bo).
                        # DVE, not ACT: during interleaved proj/attention
                        # these would queue ahead of the exps that pace the PE
                        nc.vector.tensor_copy(dst[:, dcol:dcol + 1024], ps[:])

            def emit_vbuild(b, kb0, kb1):
                # V natural per key block: v0/v1 = [V(h) | ones]; the ones
                # column accumulates the softmax denominator into psum row 64.
                if kb0 == 0:
                    st[b]["v0"] = qkvp.tile([128, _NKB, 65], f16, tag="v0",
                                            name="v0")
                    st[b]["v1"] = qkvp.tile([128, _NKB, 65], f16, tag="v1",
                                            name="v1")
                v0, v1 = st[b]["v0"], st[b]["v1"]
                vT = st[b]["vT"]
                for kb in range(kb0, kb1):
                    vt = opp.tile([128, 512], f16, tag="op", name="vt")
                    nc.tensor.transpose(
                        vt[:, 0:128], vT[:, kb * 128:(kb + 1) * 128], idn_sb[:]
                    )
                    nc.vector.tensor_copy(v0[:, kb, 0:64], vt[:, 0:64])
                    nc.vector.tensor_copy(v1[:, kb, 0:64], vt[:, 64:128])
                nc.vector.tensor_copy(v0[:, kb0:kb1, 64], onsb_sb[:, kb0:kb1])
                nc.vector.tensor_copy(v1[:, kb0:kb1, 64], onsb_sb[:, kb0:kb1])

            def emit_attn_qt(b, qt, split_tail=False):
                t0 = b * _L
                q0 = qt * 512
                nk = 4 * (qt + 1)       # causal: key blocks 0..nk-1
                qT, kT = st[b]["qT"], st[b]["kT"]
                v0, v1 = st[b]["v0"], st[b]["v1"]
                ctx_sb = st[b]["ctx"]
                ctxA = cxp.tile([128, 512], f32, tag="cx", name="ctxA")
                ctxB = cxp.tile([128, 512], f32, tag="cx", name="ctxB")
                for kb in range(nk):
                    k0 = kb * 128
                    mi = kb - (nk - 4)   # diagonal-block index, <0 off-diag
                    # queries below 128*mi of this qt are fully masked for
                    # this key block: only compute scores for the last w
                    # columns. exp/mask/ctx stay full width - the stale psum
                    # in the skipped columns exps to a finite value that the
                    # mask multiply zeroes.
                    w = 512 - 128 * mi if mi > 0 else 512
                    sc = scp.tile([128, 2, 512], f32, tag="sc")
                    for h in range(2):
                        hp = h * 64
                        nc.tensor.matmul(
                            sc[:, h, 512 - w:512],
                            kT[hp:hp + 64, k0:k0 + 128],
                            qT[hp:hp + 64, q0 + 512 - w:q0 + 512],
                            start=True, stop=True,
                        )
                    pr = probsp.tile([128, 2, 512], f16, tag="pr")
                    # exp only the live columns (strided across both heads);
                    # the full-width mask multiply zeroes the stale columns.
                    nc.scalar.activation(
                        pr[:, :, 512 - w:512], sc[:, :, 512 - w:512],
                        AFT.Exp, scale=0.125,
                    )
                    if mi >= 0:
                        nc.vector.tensor_mul(
                            pr[:, :, :], pr[:, :, :], msk_sb[:, mi, :, :]
                        )
                    nc.tensor.matmul(
                        ctxA[0:65, :], v0[:, kb, :], pr[:, 0, :],
                        start=(kb == 0), stop=(kb == nk - 1),
                    )
                    nc.tensor.matmul(
                        ctxB[0:65, :], v1[:, kb, :], pr[:, 1, :],
                        start=(kb == 0), stop=(kb == nk - 1),
                    )
                # normalize. The raw ctx psum->sbuf copies run in parallel
                # with the reciprocal/broadcast chain, so the final multiply
                # waits on neither. For the last qt the 512 queries are
                # processed in two halves so the output projection of the
                # first half overlaps the second half's normalize (shrinks
                # the end-of-kernel drain).
                segs = ((0, 256), (256, 512)) if split_tail else ((0, 512),)
                for c0, c1w in segs:
                    w = c1w - c0
                    csA = stagep.tile([64, w], f32, tag="csA", name="csA")
                    csB = stagep.tile([64, w], f32, tag="csB", name="csB")
                    nc.vector.tensor_copy(csA[:], ctxA[0:64, c0:c1w])
                    nc.vector.tensor_copy(csB[:], ctxB[0:64, c0:c1w])
                    dn = stagep.tile([65, w], f32, tag="dn", name="dn")
                    nc.scalar.copy(dn[0:1, :], ctxA[64:65, c0:c1w])
                    nc.scalar.copy(dn[64:65, :], ctxB[64:65, c0:c1w])
                    rc = stagep.tile([65, w], f32, tag="rc", name="rc")
                    # one approx reciprocal covers both denominator rows (DVE
                    # time scales with free size only; partitions 1..63
                    # compute garbage that is never read). Denominators are
                    # >= 1 so no approx edge cases.
                    nc.vector.reciprocal_approx_fast(rc[:], dn[:])
                    rcr = stagep.tile([65, w], f32r, tag="rcr", name="rcr")
                    with nc.allow_low_precision(
                        reason="f32r rounding cast feeds f32r matmul; ~1e-7 ok"
                    ):
                        nc.vector.tensor_copy(rcr[:], rc[:])
                    bcA = opp.tile([128, 512], f32, tag="op", name="bcA")
                    bcB = opp.tile([128, 512], f32, tag="op", name="bcB")
                    nc.tensor.matmul(
                        bcA[0:64, 0:w], ons_sb[0:1, :], rcr[0:1, :],
                        start=True, stop=True,
                    )
                    nc.tensor.matmul(
                        bcB[0:64, 0:w], ons_sb[64:65, :], rcr[64:65, :],
                        start=True, stop=True,
                    )
                    nc.vector.tensor_mul(
                        ctx_sb[0:64, q0 + c0:q0 + c1w], csA[:], bcA[0:64, 0:w]
                    )
                    # head 1's normalized ctx is born on partitions 0:64; a
                    # small SBUF->SBUF DMA shifts it to partitions 64:128 so
                    # the output projection contracts over all 128 dims.
                    c1 = stagep.tile([64, w], f16, tag="c1", name="c1")
                    nc.vector.tensor_mul(c1[:], csB[:], bcB[0:64, 0:w])
                    # issue on the ACT DGE queue: on Sync it would sit behind
                    # the previous qt's output DMAs, delaying this qt's
                    # output projection by the issue latency.
                    nc.scalar.dma_start(
                        ctx_sb[64:128, q0 + c0:q0 + c1w], c1[:]
                    )
                    # output projection (full-128 contraction); one staged
                    # [128,1024] f16 tile per 128 tokens.
                    def emit_op_tkb(tkb):
                        tk0 = q0 + tkb * 128
                        og = ostgp.tile([128, 1024], f16, tag="og")
                        for half in range(2):
                            oph = opp.tile(
                                [128, 512], f32, tag="op", name="oph"
                            )
                            nc.tensor.matmul(
                                oph[:],
                                ctx_sb[:, tk0:tk0 + 128],
                                wo_sb[:, half * 512:(half + 1) * 512],
                                start=True, stop=True,
                            )
                            if b == 1 and qt >= 2 and half == 1:
                                # ACT has slack in the drain; offload half the
                                # staging copies from the busier DVE
                                nc.scalar.copy(
                                    og[:, half * 512:(half + 1) * 512], oph[:]
                                )
                            else:
                                nc.vector.tensor_copy(
                                    og[:, half * 512:(half + 1) * 512], oph[:]
                                )
                            if b == 1 and qt == 3:
                                # drain: ship each half as soon as its copy
                                # lands instead of waiting for both (the Sync
                                # queue is empty here, issues are free)
                                nc.sync.dma_start(
                                    out[t0 + tk0:t0 + tk0 + 128,
                                        half * 512:(half + 1) * 512],
                                    og[:, half * 512:(half + 1) * 512],
                                )
                        if not (b == 1 and qt == 3):
                            nc.sync.dma_start(
                                out[t0 + tk0:t0 + tk0 + 128, :], og[:]
                            )

                    for tkb in range(c0 // 128, c1w // 128):
                        emit_op_tkb(tkb)

            # ---- emission: start attention as soon as its key blocks
            # exist, and interleave batch-1 projections into batch-0
            # attention so PSUM rings pipeline across phases ----
            emit_x_load(0, 0)
            emit_x_load(0, 1)
            emit_proj_strip(0, 0)
            # group-2 constants (masks, Wo) are first needed ~20us in; issuing
            # them here keeps the 8-16us DMA window clear for x strip 0,
            # which gates the first projection matmuls.
            emit_group2_consts()
            emit_vbuild(0, 0, 8)
            emit_attn_qt(0, 0)
            emit_proj_strip(0, 1)
            emit_attn_qt(0, 1)
            emit_vbuild(0, 8, 16)
            emit_attn_qt(0, 2)
            emit_x_load(1, 0)
            emit_x_load(1, 1)
            emit_proj_strip(1, 0)
            emit_attn_qt(0, 3)
            emit_proj_strip(1, 1)
            emit_vbuild(1, 0, 8)
            emit_attn_qt(1, 0)
            emit_attn_qt(1, 1)
            emit_vbuild(1, 8, 16)
            emit_attn_qt(1, 2)
            emit_attn_qt(1, 3)

    nc.compile()
    return nc


def _get_nc():
    if "nc" not in _cache:
        _cache["nc"] = _build_bass()
    return _cache["nc"]


def _wchunks(W, s):
    # [D, 128] slice -> [128(p), 8(chunk), 128(d)] contiguous for fast DMA
    w = np.asarray(W, np.float32)[:, s].astype(np.float16)
    return np.ascontiguousarray(w.reshape(8, 128, 128).transpose(1, 0, 2))


def _host_inputs(x, Wq, bq, Wk, bk, Wv, bv, Wo, bo):
    x = np.asarray(x, np.float32)
    xT = np.ascontiguousarray(x.reshape(_T, _D).T.astype(np.float16))

    # diagonal-block causal masks, duplicated for the two packed heads:
    # mask[i][k, q % 512] = 1 if (128*i + k) <= q
    kk = np.arange(128)[:, None]
    qq = np.arange(512)[None, :]
    m = np.stack([(qq >= kk + 128 * i).astype(np.float16) for i in range(4)])
    masks = np.ascontiguousarray(
        np.concatenate([m, m], axis=-1).transpose(1, 0, 2)
    ).reshape(128, 4, 2, 512)
    ident = np.eye(128, dtype=np.float16)
    ones = np.ones((128, 64), np.float32)
    onesb = np.ones((128, _NKB), np.float16)

    in_maps = []
    for c in range(_NC):
        s = slice(c * _DC, (c + 1) * _DC)
        in_maps.append({
            "xT": xT,
            "wq": _wchunks(Wq, s),
            "wk": _wchunks(Wk, s),
            "wv": _wchunks(Wv, s),
            "wo": np.ascontiguousarray(np.asarray(Wo, np.float32)[s, :].astype(np.float16)),
            "bq": np.ascontiguousarray(np.asarray(bq, np.float32)[s, None]),
            "bk": np.ascontiguousarray(np.asarray(bk, np.float32)[s, None]),
            "msk": masks,
            "idn": ident,
            "ons": ones,
            "onsb": onesb,
        })
    return in_maps


def kernel_run(x, Wq, bq, Wk, bk, Wv, bv, Wo, bo, trace=False):
    """Run the SPMD kernel; returns (full output, BassKernelResults)."""
    from concourse.bass_utils import run_bass_kernel_spmd

    nc = _get_nc()
    in_maps = _host_inputs(x, Wq, bq, Wk, bk, Wv, bv, Wo, bo)
    res = run_bass_kernel_spmd(nc, in_maps, list(range(_NC)), trace=trace)
    acc = np.zeros((_T, _D), np.float32)
    for c in range(_NC):
        acc += res.results[c]["out"].astype(np.float32)
    # softmax rows sum to 1, so the V bias contributes the constant row
    # bv @ Wo to every token; fold it into the output bias here.
    bo_eff = np.asarray(bo, np.float32) + (
        np.asarray(bv, np.float32) @ np.asarray(Wo, np.float32)
    )
    acc += bo_eff[None, :]
    return acc.reshape(_B, _L, _D), res


def kernel(x, Wq, bq, Wk, bk, Wv, bv, Wo, bo):
    out, _ = kernel_run(x, Wq, bq, Wk, bk, Wv, bv, Wo, bo, trace=False)
    return out
